# revision 1
# baseline (speedup 1.0000x reference)
"""Distributed multi-head attention kernel for 8 TRN2 NeuronCores.

Problem: x [4, 2048, 1024] -> qkv proj -> 16-head attention (d=64)
         -> out proj + bias -> [4, 2048, 1024].

Sharding (no collectives): core i handles batch b = i//2 and query-half
half = i%2 (1024 query tokens). Each core computes K/V for its batch's
full 2048-token sequence (duplicated within the pair of cores sharing a
batch) and Q only for its own 1024 tokens. The host rotates the token
axis per core so the core's query tokens are always tokens [0, 1024) of
its input -- attention is permutation-invariant over keys, so K/V token
order does not matter.

Per-core pipeline (everything bf16 on the TensorE, fp32 PSUM accum):
  proj:  Q^T [d, q] / K^T [d, k] head-pairs packed on 128 partitions;
         V [k, d] in 65-wide per-head blocks with a ones column
         (the PV matmul then yields softmax denominators for free).
  attn:  per head: S^T = K @ Q^T -> exp on ScalarE (x0.125 fused, no
         max subtraction; scores are O(1) by construction) -> bf16 P^T
         -> PV accumulation U^T[65, q]; row 64 = denominator.
         Tail: U^T -> SBUF bf16 + 1/D (fp16) immediately (frees PSUM);
         normalize = K=1 ones matmul broadcast + DVE multiply, off the
         critical path.
  out:   two passes (heads 0-7 + bias, then heads 8-15) accumulating
         through a DRAM scratch so pass A fills the PE during the
         ACT-bound attention of the second half.

The two halves' projections and attention phases are arranged so the
PE always has matmul work while the ScalarE grinds through exp()
(keeps the PE HAM clock gate at 2.4 GHz).
"""

import numpy as np
import ml_dtypes

B = 4
N = 2048
DIM = 1024
HEADS = 16
DH = 64
NQ = 1024  # query tokens per core
NCORES = 8

_CACHE = {}


def _build_nc():
    from contextlib import ExitStack

    import concourse.bass as bass
    import concourse.mybir as mybir
    import concourse.tile as tile
    from concourse import bacc

    f32 = mybir.dt.float32
    bf16 = mybir.dt.bfloat16
    f16 = mybir.dt.float16
    EXP = mybir.ActivationFunctionType.Exp

    nc = bacc.Bacc("TRN2", target_bir_lowering=False, debug=False,
                   num_devices=NCORES)

    xt_d = nc.dram_tensor("xt", [DIM, N], bf16, kind="ExternalInput")
    wqkv_d = nc.dram_tensor("wqkv", [DIM, 3 * DIM], bf16, kind="ExternalInput")
    wo_d = nc.dram_tensor("wo", [HEADS, DH, DIM], bf16, kind="ExternalInput")
    brow_d = nc.dram_tensor("brow", [1, DIM], bf16, kind="ExternalInput")
    out_d = nc.dram_tensor("out", [NQ, DIM], f32, kind="ExternalOutput")

    with tile.TileContext(nc) as tc, ExitStack() as top:
        const_pool = top.enter_context(tc.tile_pool(name="const", bufs=1))
        mm_psum = top.enter_context(tc.tile_pool(name="mmps", bufs=2, space="PSUM"))
        sp_psum = top.enter_context(tc.tile_pool(name="spps", bufs=2, space="PSUM"))
        u_psum = top.enter_context(tc.tile_pool(name="ups", bufs=1, space="PSUM"))
        es_pool = top.enter_context(tc.tile_pool(name="es", bufs=6))
        rec_pool = top.enter_context(tc.tile_pool(name="rec", bufs=4))
        bc_pool = top.enter_context(tc.tile_pool(name="bc", bufs=3))
        uraw_a = top.enter_context(tc.tile_pool(name="uraw_a", bufs=1))
        dram_pool = top.enter_context(tc.tile_pool(name="dscr", bufs=1, space="DRAM"))

        brow_t = const_pool.tile([1, DIM], bf16, tag="brow", name="brow")
        nc.sync.dma_start(brow_t[:], brow_d.ap()[:])
        ones_t = const_pool.tile([1, 128], bf16, tag="ones", name="ones")
        nc.gpsimd.memset(ones_t[:], 1.0)
        ones_bc = const_pool.tile([128, 64], f16, tag="ones_bc", name="ones_bc")
        nc.gpsimd.memset(ones_bc[:], 1.0)

        uraw = [None] * HEADS

        def proj_units(half, w_pool, xt, QT, KT, VT):
            """Emission closures, one PSUM-group each.

            Order: all of V, then K/Q alternating per head-pair chunk so
            early head pairs become ready as soon as possible.
            """
            def dma_factory(col0):
                box = [None]
                def dma():
                    if box[0] is None:
                        wb = [w_pool.tile([128, 512], bf16, tag=f"w{fc}",
                                          name=f"w{fc}") for fc in range(8)]
                        for fc in range(8):
                            nc.sync.dma_start(
                                wb[fc][:],
                                wqkv_d.ap()[fc * 128:(fc + 1) * 128,
                                            col0:col0 + 512])
                        box[0] = wb
                    return box[0]
                return dma

            dma_v = dma_factory(2 * DIM + half * 512)
            dma_k = dma_factory(DIM + half * 512)
            dma_q = dma_factory(half * 512)

            def v_unit(mk, dma=dma_v):
                wb = dma()
                ps = mm_psum.tile([128, 512], f32, tag="mm", name="mm")
                for fc in range(8):
                    nc.tensor.matmul(
                        ps[:], xt[fc][:, mk * 128:(mk + 1) * 128], wb[fc][:],
                        start=(fc == 0), stop=(fc == 7))
                nc.vector.tensor_copy(
                    VT[mk][:, :, 0:64],
                    ps[:].rearrange("p (h d) -> p h d", d=64))
                nc.gpsimd.memset(VT[mk][:, :, 64:65], 1.0)

            def qk_unit(dma, dest, m4, t):
                wb = dma()
                ps = mm_psum.tile([128, 512], f32, tag="mm", name="mm")
                for fc in range(8):
                    nc.tensor.matmul(
                        ps[:], wb[fc][:, m4 * 128:(m4 + 1) * 128],
                        xt[fc][:, t * 512:(t + 1) * 512],
                        start=(fc == 0), stop=(fc == 7))
                nc.vector.tensor_copy(
                    dest[m4][:, t * 512:(t + 1) * 512], ps[:])

            units = [lambda mk=mk: v_unit(mk) for mk in range(16)]
            for m4 in range(4):
                for t in range(4):
                    units.append(lambda m4=m4, t=t: qk_unit(dma_k, KT, m4, t))
                for t in range(2):
                    units.append(lambda m4=m4, t=t: qk_unit(dma_q, QT, m4, t))
            return units

        def emit_norm(h):
            """Normalize head h's raw U by its softmax denominators."""
            rec = _recs.pop(h)
            for qc in range(2):
                bc = mm_psum.tile([64, 512], f32, tag="mm", name="bc")
                nc.tensor.matmul(
                    bc[:], ones_bc[64:65, :],
                    rec[64:65, qc * 512:(qc + 1) * 512],
                    start=True, stop=True)
                bc_sb = bc_pool.tile([64, 512], f32, tag="bc", name="bc_sb")
                nc.vector.tensor_copy(bc_sb[:], bc[:])
                nc.gpsimd.tensor_mul(
                    uraw[h][0:64, qc * 512:(qc + 1) * 512],
                    uraw[h][0:64, qc * 512:(qc + 1) * 512], bc_sb[:])

        _recs = {}

        def emit_attn(heads, QTs, KTs, VTs, uraw_pools, fillers,
                      hooks=None):
            """Attention for the given heads; filler units spread across
            k-steps. Normalize for head h is emitted one head late."""
            fillers = list(fillers)
            nfill = len(fillers)
            steps = len(heads) * 16
            done = 0
            for hh_i, h in enumerate(heads):
                if hooks and h in hooks:
                    hooks[h]()
                half = h // 8
                hh = h % 8
                QT, KT, VT = QTs[half], KTs[half], VTs[half]
                pair = hh // 2
                hb = (hh % 2) * 64
                Ups = u_psum.tile([65, 2, 512], f32, tag="up", name="up")
                for k in range(16):
                    sp = sp_psum.tile([128, 2, 512], f32, tag="sp", name="sp")
                    for qc in range(2):
                        nc.tensor.matmul(
                            sp[:, qc, :],
                            KT[pair][hb:hb + 64, k * 128:(k + 1) * 128],
                            QT[pair][hb:hb + 64, qc * 512:(qc + 1) * 512],
                            start=True, stop=True)
                    es = es_pool.tile([128, 2, 512], bf16, tag="es", name="es")
                    nc.scalar.activation(es[:], sp[:], EXP, scale=0.125)
                    for qc in range(2):
                        nc.tensor.matmul(
                            Ups[:, qc, :],
                            VT[k][:, hh, :],
                            es[:, qc, :],
                            start=(k == 0), stop=(k == 15))
                    done += 1
                    while fillers and (nfill - len(fillers)) * steps < done * nfill:
                        fillers.pop(0)()
                # free the PSUM slot fast: one copy stashes raw U + D
                ur = uraw_pools[half]().tile([65, NQ], bf16, tag=f"uraw{h}",
                                             name=f"uraw{h}")
                uraw[h] = ur
                nc.vector.tensor_copy(
                    ur[:].rearrange("p (a b) -> p a b", a=2), Ups[:, :, :])
                # slow reciprocal runs from SBUF, off the PSUM critical chain
                rec = rec_pool.tile([65, NQ], f16, tag="rec", name="rec")
                with nc.allow_low_precision(reason="softmax denom recip fp16"):
                    nc.vector.reciprocal(
                        rec[64:65, :], ur[64:65, :])
                _recs[h] = rec
                if hh_i > 0:
                    emit_norm(heads[hh_i - 1])
            emit_norm(heads[-1])
            for f in fillers:
                f()

        # ---------------- emission ----------------
        if True:
            xt_pool = tc.alloc_tile_pool(name="xt", bufs=1)
            w_pool = tc.alloc_tile_pool(name="w", bufs=2)
            xt = [xt_pool.tile([128, N], bf16, tag=f"xt{i}", name=f"xt{i}")
                  for i in range(8)]
            for i in range(8):
                nc.sync.dma_start(xt[i][:], xt_d.ap()[i * 128:(i + 1) * 128, :])

            qkv0 = tc.alloc_tile_pool(name="qkv0", bufs=1)
            QT0 = [qkv0.tile([128, NQ], bf16, tag=f"q{m}", name=f"q0{m}")
                   for m in range(4)]
            KT0 = [qkv0.tile([128, N], bf16, tag=f"k{m}", name=f"k0{m}")
                   for m in range(4)]
            VT0 = [qkv0.tile([128, 8, 65], bf16, tag=f"v{mk}", name=f"v0{mk}")
                   for mk in range(16)]
            p0_units = proj_units(0, w_pool, xt, QT0, KT0, VT0)
            # V + pair-0 K/Q serially (heads 0/1 cannot start without them;
            # Tile dependencies only look backward in emission order)
            for c in p0_units[:22]:
                c()
            p0_rest = p0_units[22:]

            qkv1 = tc.alloc_tile_pool(name="qkv1", bufs=1, side="right")
            QT1 = [qkv1.tile([128, NQ], bf16, tag=f"q{m}", name=f"q1{m}")
                   for m in range(4)]
            KT1 = [qkv1.tile([128, N], bf16, tag=f"k{m}", name=f"k1{m}")
                   for m in range(4)]
            VT1 = [qkv1.tile([128, 8, 65], bf16, tag=f"v{mk}", name=f"v1{mk}")
                   for mk in range(16)]
            p1_units = proj_units(1, w_pool, xt, QT1, KT1, VT1)

            state = {}

            def setup_b():
                qkv0.release()
                state["uraw_b"] = tc.alloc_tile_pool(name="uraw_b", bufs=1,
                                                     side="right")
                wo_pool = tc.alloc_tile_pool(name="wo", bufs=1, side="right")
                state["wo_pool"] = wo_pool
                state["WO"] = [wo_pool.tile([64, DIM], bf16, tag=f"wo{h}",
                                            name=f"wo{h}")
                               for h in range(HEADS)]
                for h in range(HEADS):
                    nc.sync.dma_start(state["WO"][h][:], wo_d.ap()[h])


            def setup_c():
                # xt and the w-block tiles are dead once proj(1) is done
                w_pool.release()
                xt_pool.release()
                state["st_pool"] = tc.alloc_tile_pool(name="st", bufs=2)
                state["FIN"] = [
                    state["st_pool"].tile([128, DIM], f32, tag=f"fin{qf}",
                                          name=f"fin{qf}", bufs=1)
                    for qf in range(8)]

            # pass A unit: heads 0-7 + bias for one qf -> resident FIN tile
            def passA(qf):
                WO = state["WO"]
                fin = state["FIN"][qf]
                for of in range(2):
                    ps = mm_psum.tile([128, 512], f32, tag="mm", name="mm")
                    for hh in range(8):
                        nc.tensor.matmul(
                            ps[:],
                            uraw[hh][0:64, qf * 128:(qf + 1) * 128],
                            WO[hh][:, of * 512:(of + 1) * 512],
                            start=(hh == 0), stop=False)
                    nc.tensor.matmul(
                        ps[:], ones_t[:, 0:128],
                        brow_t[:, of * 512:(of + 1) * 512],
                        start=False, stop=True)
                    nc.vector.tensor_copy(fin[:, of * 512:(of + 1) * 512],
                                          ps[:])

            uraw_pools = {0: lambda: uraw_a, 1: lambda: state["uraw_b"]}

            # heads 0-9: remaining proj0 + all proj1 units fill PE gaps
            emit_attn(range(0, 10), [QT0, QT1], [KT0, KT1], [VT0, VT1],
                      uraw_pools, p0_rest + p1_units[:34],
                      hooks={8: setup_b})
            # heads 10-15: pass A units fill PE gaps
            emit_attn(range(10, 16), [QT0, QT1], [KT0, KT1], [VT0, VT1],
                      uraw_pools,
                      p1_units[34:] +
                      [lambda qf=qf: passA(qf) for qf in range(8)],
                      hooks={10: setup_c})

            # pass B: heads 8-15 onto the resident partials -> out.
            # h15 (the norm-gated head) goes FIRST in each accumulation
            # group so the PE pays its normalize wait once, then streams.
            for qf in range(8):
                fin = state["FIN"][qf]
                for of in range(2):
                    ps = mm_psum.tile([128, 512], f32, tag="mm", name="mm")
                    for hh in [15] + list(range(8, 15)):
                        nc.tensor.matmul(
                            ps[:],
                            uraw[hh][0:64, qf * 128:(qf + 1) * 128],
                            state["WO"][hh][:, of * 512:(of + 1) * 512],
                            start=(hh == 15), stop=(hh == 14))
                    nc.vector.tensor_add(
                        fin[:, of * 512:(of + 1) * 512],
                        fin[:, of * 512:(of + 1) * 512], ps[:])
                nc.sync.dma_start(out_d.ap()[qf * 128:(qf + 1) * 128, :], fin[:])

            state["st_pool"].release()
            state["wo_pool"].release()
            state["uraw_b"].release()
            qkv1.release()

    nc.compile()
    return nc


def _get_nc():
    if "nc" not in _CACHE:
        _CACHE["nc"] = _build_nc()
    return _CACHE["nc"]


def _make_in_maps(x, w_qkv, w_out, b_out):
    bf = ml_dtypes.bfloat16
    wo = np.ascontiguousarray(w_out.reshape(HEADS, DH, DIM)).astype(bf)
    brow = np.asarray(b_out, np.float32).reshape(1, DIM).astype(bf)
    wqkv = np.ascontiguousarray(w_qkv, np.float32).astype(bf)
    in_maps = []
    for i in range(NCORES):
        b, half = i // 2, i % 2
        xt = np.asarray(x[b], np.float32).T.astype(bf)  # [DIM, N]
        if half:
            xt = np.concatenate([xt[:, NQ:], xt[:, :NQ]], axis=1)
        in_maps.append({
            "xt": np.ascontiguousarray(xt),
            "wqkv": wqkv,
            "wo": wo,
            "brow": brow,
        })
    return in_maps


def _assemble(results):
    out = np.empty((B, N, DIM), np.float32)
    for i in range(NCORES):
        b, half = i // 2, i % 2
        out[b, half * NQ:(half + 1) * NQ, :] = results[i]["out"]
    return out


def run(x, w_qkv, w_out, b_out, trace=False):
    """Run the kernel; returns (output, BassKernelResults)."""
    from concourse.bass_utils import run_bass_kernel_spmd
    nc = _get_nc()
    in_maps = _make_in_maps(x, w_qkv, w_out, b_out)
    res = run_bass_kernel_spmd(nc, in_maps, core_ids=list(range(NCORES)),
                               trace=trace)
    return _assemble(res.results), res


def kernel(x, w_qkv, w_out, b_out):
    out, _ = run(x, w_qkv, w_out, b_out, trace=False)
    return out



# revision 7
# speedup vs baseline: 1.0396x; 1.0396x over previous
"""Distributed multi-head attention kernel for 8 TRN2 NeuronCores.

Problem: x [4, 2048, 1024] -> qkv proj -> 16-head attention (d=64)
         -> out proj + bias -> [4, 2048, 1024].

Sharding (no collectives): core i handles batch b = i//2 and query-half
half = i%2 (1024 query tokens). Each core computes K/V for its batch's
full 2048-token sequence (duplicated within the pair of cores sharing a
batch) and Q only for its own 1024 tokens. The host rotates the token
axis per core so the core's query tokens are always tokens [0, 1024) of
its input -- attention is permutation-invariant over keys, so K/V token
order does not matter.

Per-core pipeline (bf16 on the TensorE, fp32 PSUM accum):
  proj:  Q^T [d, q] / K^T [d, k] head-pairs packed on 128 partitions;
         V split even/odd-head: even heads [64 V | ones] (denominator in
         PSUM row 64), odd heads [ones | zeros | 64 V] so U lands on
         PSUM rows 64-127 and D on row 0.  A head-pair's U is thus
         assembled pair-stacked in SBUF [128, q] which makes the out
         projection a full contract-128 matmul (half the instructions).
  attn:  per head: S^T = K @ Q^T -> exp on ScalarE (x0.125 fused; no max
         subtraction, scores are O(1)) -> bf16 P^T -> PV accumulation.
         Denominator rows are staged via ScalarE copies + SBUF-to-SBUF
         DMA onto partitions {0,32,64,96} and reciprocal'd once per
         4-head group (the DVE reciprocal is free-dim-bound at 8
         cycles/elem, so batching heads onto partitions is 4x cheaper).
  out:   three passes (pairs 0-3 + bias, pairs 4-5, pairs 6-7)
         accumulating into resident SBUF tiles, so most of the out
         projection fills PE gaps during ScalarE-bound attention and the
         tail after the last head is short.
"""

import numpy as np
import ml_dtypes

B = 4
N = 2048
DIM = 1024
HEADS = 16
DH = 64
NQ = 1024  # query tokens per core
NCORES = 8

_CACHE = {}


def _build_nc():
    from contextlib import ExitStack

    import concourse.bass as bass
    import concourse.mybir as mybir
    import concourse.tile as tile
    from concourse import bacc

    f32 = mybir.dt.float32
    bf16 = mybir.dt.bfloat16
    EXP = mybir.ActivationFunctionType.Exp

    nc = bacc.Bacc("TRN2", target_bir_lowering=False, debug=False,
                   num_devices=NCORES)

    xt_d = nc.dram_tensor("xt", [DIM, N], bf16, kind="ExternalInput")
    wqkv_d = nc.dram_tensor("wqkv", [DIM, 3 * DIM], bf16, kind="ExternalInput")
    wo_d = nc.dram_tensor("wo", [HEADS // 2, 2 * DH, DIM], bf16,
                          kind="ExternalInput")
    brow_d = nc.dram_tensor("brow", [1, DIM], bf16, kind="ExternalInput")
    out_d = nc.dram_tensor("out", [NQ, DIM], f32, kind="ExternalOutput")

    with tile.TileContext(nc) as tc, ExitStack() as top:
        const_pool = top.enter_context(tc.tile_pool(name="const", bufs=1))
        mm_psum = top.enter_context(tc.tile_pool(name="mmps", bufs=2, space="PSUM"))
        sp_psum = top.enter_context(tc.tile_pool(name="spps", bufs=2, space="PSUM"))
        u_psum = top.enter_context(tc.tile_pool(name="ups", bufs=1, space="PSUM"))
        es_pool = top.enter_context(tc.tile_pool(name="es", bufs=6))
        dst_pool = top.enter_context(tc.tile_pool(name="dst", bufs=2))
        bc_pool = top.enter_context(tc.tile_pool(name="bc", bufs=3))
        uraw_a = top.enter_context(tc.tile_pool(name="uraw_a", bufs=1))

        brow_t = const_pool.tile([1, DIM], bf16, tag="brow", name="brow")
        nc.sync.dma_start(brow_t[:], brow_d.ap()[:])
        ones_t = const_pool.tile([1, 128], bf16, tag="ones", name="ones")
        nc.gpsimd.memset(ones_t[:], 1.0)
        # all-ones rows used by the per-pair normalizer broadcast matmuls;
        # only partitions {0,32,64,96} are used (contract-1 matmul rule).
        onesf = const_pool.tile([97, 128], f32, tag="onesf", name="onesf")
        nc.gpsimd.memset(onesf[:], 1.0)
        # denominator staging: head 4g+j's D row lands on partition 32*j.
        dsb_t = const_pool.tile([97, NQ], bf16, tag="dsb", name="dsb")
        rec_t = const_pool.tile([97, NQ], f32, tag="rec", name="rec")

        ur2 = [None] * (HEADS // 2)  # pair-stacked raw U [128, NQ] bf16

        def proj_units(half, w_pool, xt, QT, KT, VTe, VTo):
            """Emission closures, one PSUM-group each.

            Order: all of V, then K/Q alternating per head-pair chunk so
            early head pairs become ready as soon as possible.
            """
            def dma_factory(col0):
                box = [None]
                def dma():
                    if box[0] is None:
                        wb = [w_pool.tile([128, 512], bf16, tag=f"w{fc}",
                                          name=f"w{fc}") for fc in range(8)]
                        for fc in range(8):
                            nc.sync.dma_start(
                                wb[fc][:],
                                wqkv_d.ap()[fc * 128:(fc + 1) * 128,
                                            col0:col0 + 512])
                        box[0] = wb
                    return box[0]
                return dma

            dma_v = dma_factory(2 * DIM + half * 512)
            dma_k = dma_factory(DIM + half * 512)
            dma_q = dma_factory(half * 512)

            def v_unit(mk, dma=dma_v):
                wb = dma()
                ps = mm_psum.tile([128, 512], f32, tag="mm", name="mm")
                for fc in range(8):
                    nc.tensor.matmul(
                        ps[:], xt[fc][:, mk * 128:(mk + 1) * 128], wb[fc][:],
                        start=(fc == 0), stop=(fc == 7))
                r = ps[:].rearrange("p (s two d) -> p s two d", two=2, d=64)
                nc.vector.tensor_copy(VTe[mk][:, :, 0:64], r[:, :, 0, :])
                nc.vector.tensor_copy(VTo[mk][:, :, 64:128], r[:, :, 1, :])

            def qk_unit(dma, dest, m4, t):
                wb = dma()
                ps = mm_psum.tile([128, 512], f32, tag="mm", name="mm")
                for fc in range(8):
                    nc.tensor.matmul(
                        ps[:], wb[fc][:, m4 * 128:(m4 + 1) * 128],
                        xt[fc][:, t * 512:(t + 1) * 512],
                        start=(fc == 0), stop=(fc == 7))
                nc.vector.tensor_copy(
                    dest[m4][:, t * 512:(t + 1) * 512], ps[:])

            units = [lambda mk=mk: v_unit(mk) for mk in range(16)]
            for m4 in range(4):
                for t in range(4):
                    units.append(lambda m4=m4, t=t: qk_unit(dma_k, KT, m4, t))
                for t in range(2):
                    units.append(lambda m4=m4, t=t: qk_unit(dma_q, QT, m4, t))
            return units, dma_v

        def emit_group_norm(g):
            """Reciprocal + normalize for the 4 heads of group g.

            Dsb rows {0,32,64,96} hold heads 4g..4g+3's denominators;
            one whole-tile reciprocal covers all four, then per pair a
            broadcast matmul spreads 1/D across the pair-stacked rows of
            ur2 and a GpSimd multiply normalizes in place.
            """
            with nc.allow_low_precision(reason="softmax denom recip"):
                nc.vector.reciprocal(rec_t[:], dsb_t[:])
            for pp in range(2):
                p = 2 * g + pp
                re_row = 32 * (2 * pp)      # even head's recst partition
                ro_row = 32 * (2 * pp + 1)  # odd head's
                for qc in range(2):
                    bc = mm_psum.tile([128, 512], f32, tag="mm", name="bc")
                    nc.tensor.matmul(
                        bc[0:64, :], onesf[re_row:re_row + 1, 0:64],
                        rec_t[re_row:re_row + 1, qc * 512:(qc + 1) * 512],
                        start=True, stop=True,
                        tile_position=(re_row, 0))
                    nc.tensor.matmul(
                        bc[64:128, :], onesf[ro_row:ro_row + 1, 64:128],
                        rec_t[ro_row:ro_row + 1, qc * 512:(qc + 1) * 512],
                        start=True, stop=True,
                        tile_position=(ro_row, 64))
                    bc_sb = bc_pool.tile([128, 512], f32, tag="bc", name="bc_sb")
                    nc.vector.tensor_copy(bc_sb[:], bc[:])
                    nc.gpsimd.tensor_mul(
                        ur2[p][:, qc * 512:(qc + 1) * 512],
                        ur2[p][:, qc * 512:(qc + 1) * 512], bc_sb[:])

        dstate = {}

        def emit_attn(heads, QTs, KTs, VTes, VTos, ur2_pool, fillers,
                      hooks=None):
            """Attention for the given heads; filler units spread across
            k-steps. Group g's normalize is emitted at head 4g+4."""
            fillers = list(fillers)
            nfill = len(fillers)
            steps = len(heads) * 16
            done = 0
            for h in heads:
                if hooks and h in hooks:
                    hooks[h]()
                if h % 4 == 0 and h > 0:
                    emit_group_norm(h // 4 - 1)
                half = h // 8
                hh = h % 8
                QT, KT = QTs[half], KTs[half]
                pair = hh // 2
                hb = (hh % 2) * 64
                odd = h % 2
                slot = hh // 2
                Ups = u_psum.tile([128, 2, 512], f32, tag="up", name="up")
                for k in range(16):
                    sp = sp_psum.tile([128, 2, 512], f32, tag="sp", name="sp")
                    for qc in range(2):
                        nc.tensor.matmul(
                            sp[:, qc, :],
                            KT[pair][hb:hb + 64, k * 128:(k + 1) * 128],
                            QT[pair][hb:hb + 64, qc * 512:(qc + 1) * 512],
                            start=True, stop=True)
                    es = es_pool.tile([128, 2, 512], bf16, tag="es", name="es")
                    nc.scalar.activation(es[:], sp[:], EXP, scale=0.125)
                    for qc in range(2):
                        if odd:
                            nc.tensor.matmul(
                                Ups[:, qc, :],
                                VTos[half][k][:, slot, :],
                                es[:, qc, :],
                                start=(k == 0), stop=(k == 15))
                        else:
                            nc.tensor.matmul(
                                Ups[0:65, qc, :],
                                VTes[half][k][:, slot, :],
                                es[:, qc, :],
                                start=(k == 0), stop=(k == 15))
                    done += 1
                    while fillers and (nfill - len(fillers)) * steps < done * nfill:
                        fillers.pop(0)()
                # stash raw U into the pair-stacked SBUF tile (frees PSUM),
                # extract the denominator row via ScalarE, stage it for the
                # batched reciprocal via SBUF-to-SBUF DMA.
                p = h // 2
                if h % 4 == 0:
                    dstate["dst"] = dst_pool.tile([65, 2, NQ], bf16,
                                                  tag="dst", name="dst")
                dst = dstate["dst"]
                pslot = p % 2
                if ur2[p] is None:
                    ur2[p] = ur2_pool(h).tile([128, NQ], bf16, tag=f"ur{p}",
                                              name=f"ur{p}")
                if odd:
                    nc.vector.tensor_copy(
                        ur2[p][64:128, :].rearrange("p (a b) -> p a b", a=2),
                        Ups[64:128, :, :])
                    nc.scalar.copy(
                        dst[0:1, pslot, :].rearrange("p (a b) -> p a b", a=2),
                        Ups[0:1, :, :])
                    nc.sync.dma_start(
                        dsb_t[32 * (2 * pslot + 1):32 * (2 * pslot + 1) + 1, :],
                        dst[0:1, pslot, :])
                else:
                    nc.vector.tensor_copy(
                        ur2[p][0:64, :].rearrange("p (a b) -> p a b", a=2),
                        Ups[0:64, :, :])
                    nc.scalar.copy(
                        dst[64:65, pslot, :].rearrange("p (a b) -> p a b", a=2),
                        Ups[64:65, :, :])
                    nc.sync.dma_start(
                        dsb_t[32 * (2 * pslot):32 * (2 * pslot) + 1, :],
                        dst[64:65, pslot, :])
            for f in fillers:
                f()

        # ---------------- emission ----------------
        if True:
            xt_pool = tc.alloc_tile_pool(name="xt", bufs=1)
            w_pool = tc.alloc_tile_pool(name="w", bufs=2)
            xt = [xt_pool.tile([128, N], bf16, tag=f"xt{i}", name=f"xt{i}")
                  for i in range(8)]

            qkv0 = tc.alloc_tile_pool(name="qkv0", bufs=1)
            QT0 = [qkv0.tile([128, NQ], bf16, tag=f"q{m}", name=f"q0{m}")
                   for m in range(4)]
            KT0 = [qkv0.tile([128, N], bf16, tag=f"k{m}", name=f"k0{m}")
                   for m in range(4)]
            VTe0 = [qkv0.tile([128, 4, 65], bf16, tag=f"ve{mk}", name=f"ve0{mk}")
                    for mk in range(16)]
            VTo0 = [qkv0.tile([128, 4, 128], bf16, tag=f"vo{mk}", name=f"vo0{mk}")
                    for mk in range(16)]
            p0_units, p0_dma_v = proj_units(0, w_pool, xt, QT0, KT0, VTe0, VTo0)

            # weight blocks for proj-0 V first (small), then the x tiles in
            # token slices so the first v_unit only waits ~1.2 MB of DMA.
            p0_dma_v()
            for i in range(8):
                nc.sync.dma_start(xt[i][:, 0:128], xt_d.ap()[i * 128:(i + 1) * 128, 0:128])
            for i in range(8):
                nc.sync.dma_start(xt[i][:, 128:1024],
                                  xt_d.ap()[i * 128:(i + 1) * 128, 128:1024])
            for i in range(8):
                nc.sync.dma_start(xt[i][:, 1024:2048],
                                  xt_d.ap()[i * 128:(i + 1) * 128, 1024:2048])

            # stationary layouts for the V matmuls: even heads carry the
            # softmax-denominator ones column at 64; odd heads put ones at
            # col 0 and V at 64-127 (U lands pair-stacked, D on row 0).
            for mk in range(16):
                nc.gpsimd.memset(VTe0[mk][:, :, 64:65], 1.0)
                nc.gpsimd.memset(VTo0[mk][:, :, 0:64], 0.0)
                nc.gpsimd.memset(VTo0[mk][:, :, 0:1], 1.0)

            # V + pair-0 K/Q serially (heads 0/1 cannot start without them;
            # Tile dependencies only look backward in emission order)
            for c in p0_units[:22]:
                c()
            p0_rest = p0_units[22:]

            qkv1 = tc.alloc_tile_pool(name="qkv1", bufs=1, side="right")
            QT1 = [qkv1.tile([128, NQ], bf16, tag=f"q{m}", name=f"q1{m}")
                   for m in range(4)]
            KT1 = [qkv1.tile([128, N], bf16, tag=f"k{m}", name=f"k1{m}")
                   for m in range(4)]
            VTe1 = [qkv1.tile([128, 4, 65], bf16, tag=f"ve{mk}", name=f"ve1{mk}")
                    for mk in range(16)]
            VTo1 = [qkv1.tile([128, 4, 128], bf16, tag=f"vo{mk}", name=f"vo1{mk}")
                    for mk in range(16)]
            for mk in range(16):
                nc.gpsimd.memset(VTe1[mk][:, :, 64:65], 1.0)
                nc.gpsimd.memset(VTo1[mk][:, :, 0:64], 0.0)
                nc.gpsimd.memset(VTo1[mk][:, :, 0:1], 1.0)
            p1_units, _ = proj_units(1, w_pool, xt, QT1, KT1, VTe1, VTo1)

            state = {}

            def setup_b():
                qkv0.release()
                state["uraw_b"] = tc.alloc_tile_pool(name="uraw_b", bufs=1,
                                                     side="right")
                wo_pool = tc.alloc_tile_pool(name="wo", bufs=1, side="right")
                state["wo_pool"] = wo_pool
                state["WO2"] = [wo_pool.tile([128, DIM], bf16, tag=f"wo{p}",
                                             name=f"wo{p}")
                                for p in range(HEADS // 2)]
                for p in range(HEADS // 2):
                    nc.sync.dma_start(state["WO2"][p][:], wo_d.ap()[p])

            def setup_c():
                # xt and the w-block tiles are dead once proj(1) is done
                w_pool.release()
                xt_pool.release()
                state["st_pool"] = tc.alloc_tile_pool(name="st", bufs=2)
                state["FIN"] = [
                    state["st_pool"].tile([128, DIM], f32, tag=f"fin{qf}",
                                          name=f"fin{qf}", bufs=1)
                    for qf in range(8)]

            # out-proj pass over `pairs` for one qf; first pass also folds
            # in the bias row and establishes the resident FIN tile.
            def out_pass(qf, pairs, first):
                WO2 = state["WO2"]
                fin = state["FIN"][qf]
                ps = [mm_psum.tile([128, 512], f32, tag="mm", name="mm")
                      for _ in range(2)]
                for i, p in enumerate(pairs):
                    last = (i == len(pairs) - 1) and not first
                    for of in range(2):
                        nc.tensor.matmul(
                            ps[of][:],
                            ur2[p][:, qf * 128:(qf + 1) * 128],
                            WO2[p][:, of * 512:(of + 1) * 512],
                            start=(i == 0), stop=last)
                if first:
                    # fold the bias row in as the accumulation-group closer
                    for of in range(2):
                        nc.tensor.matmul(
                            ps[of][:], ones_t[:, 0:128],
                            brow_t[:, of * 512:(of + 1) * 512],
                            start=False, stop=True)
                for of in range(2):
                    if first:
                        nc.vector.tensor_copy(fin[:, of * 512:(of + 1) * 512],
                                              ps[of][:])
                    else:
                        nc.vector.tensor_add(
                            fin[:, of * 512:(of + 1) * 512],
                            fin[:, of * 512:(of + 1) * 512], ps[of][:])

            ur2_pool = lambda h: (uraw_a if h < 8 else state["uraw_b"])

            # heads 0-9: remaining proj0 + all proj1 units fill PE gaps
            emit_attn(range(0, 10), [QT0, QT1], [KT0, KT1],
                      [VTe0, VTe1], [VTo0, VTo1], ur2_pool,
                      p0_rest + p1_units[:34], hooks={8: setup_b})
            # heads 10-13: rest of proj1 + pass A (pairs 0-3, needs norm g0+g1
            # which are emitted at heads 8 and 12... g1 at head 12) -- pass A
            # only needs pairs 0-3 normalized, i.e. norm g0 (head 8) and norm
            # g1 (head 12).  Emit pass A for heads 12-13's steps.
            emit_attn(range(10, 14), [QT0, QT1], [KT0, KT1],
                      [VTe0, VTe1], [VTo0, VTo1], ur2_pool,
                      p1_units[34:] +
                      [lambda qf=qf: out_pass(qf, [0, 1, 2, 3], True)
                       for qf in range(8)],
                      hooks={10: setup_c})
            # heads 14-15: pass B (pairs 4-5; norm g2 emitted at head 14... )
            emit_attn(range(14, 16), [QT0, QT1], [KT0, KT1],
                      [VTe0, VTe1], [VTo0, VTo1], ur2_pool,
                      [lambda qf=qf: out_pass(qf, [4, 5], False)
                       for qf in range(8)])

            # tail: norm for the last group, then pass C + writeback.
            emit_group_norm(3)
            for qf in range(8):
                out_pass(qf, [6, 7], False)
                nc.sync.dma_start(out_d.ap()[qf * 128:(qf + 1) * 128, :],
                                  state["FIN"][qf][:])

            state["st_pool"].release()
            state["wo_pool"].release()
            state["uraw_b"].release()
            qkv1.release()

    nc.compile()
    return nc


def _get_nc():
    if "nc" not in _CACHE:
        _CACHE["nc"] = _build_nc()
    return _CACHE["nc"]


def _make_in_maps(x, w_qkv, w_out, b_out):
    bf = ml_dtypes.bfloat16
    wo = np.ascontiguousarray(
        w_out.reshape(HEADS // 2, 2 * DH, DIM)).astype(bf)
    brow = np.asarray(b_out, np.float32).reshape(1, DIM).astype(bf)
    wqkv = np.ascontiguousarray(w_qkv, np.float32).astype(bf)
    in_maps = []
    for i in range(NCORES):
        b, half = i // 2, i % 2
        xt = np.asarray(x[b], np.float32).T.astype(bf)  # [DIM, N]
        if half:
            xt = np.concatenate([xt[:, NQ:], xt[:, :NQ]], axis=1)
        in_maps.append({
            "xt": np.ascontiguousarray(xt),
            "wqkv": wqkv,
            "wo": wo,
            "brow": brow,
        })
    return in_maps


def _assemble(results):
    out = np.empty((B, N, DIM), np.float32)
    for i in range(NCORES):
        b, half = i // 2, i % 2
        out[b, half * NQ:(half + 1) * NQ, :] = results[i]["out"]
    return out


def run(x, w_qkv, w_out, b_out, trace=False):
    """Run the kernel; returns (output, BassKernelResults)."""
    from concourse.bass_utils import run_bass_kernel_spmd
    nc = _get_nc()
    in_maps = _make_in_maps(x, w_qkv, w_out, b_out)
    res = run_bass_kernel_spmd(nc, in_maps, core_ids=list(range(NCORES)),
                               trace=trace)
    return _assemble(res.results), res


def kernel(x, w_qkv, w_out, b_out):
    out, _ = run(x, w_qkv, w_out, b_out, trace=False)
    return out


# revision 14
# speedup vs baseline: 1.1125x; 1.0701x over previous
"""Distributed multi-head attention kernel for 8 TRN2 NeuronCores.

Problem: x [4, 2048, 1024] -> qkv proj -> 16-head attention (d=64)
         -> out proj + bias -> [4, 2048, 1024].

Sharding (no collectives): core i handles batch b = i//2 and query-half
half = i%2 (1024 query tokens). Each core computes K/V for its batch's
full 2048-token sequence (duplicated within the pair of cores sharing a
batch) and Q only for its own 1024 tokens. The host rotates the token
axis per core so the core's query tokens are always tokens [0, 1024) of
its input -- attention is permutation-invariant over keys, so K/V token
order does not matter.

Per-core pipeline (bf16 on the TensorE, fp32 PSUM accum):
  proj:  Q^T [d, q] / K^T [d, k] head-pairs packed on 128 partitions;
         V split even/odd-head: even heads [64 V | ones] (denominator in
         PSUM row 64), odd heads [ones | zeros | 64 V] so U lands on
         PSUM rows 64-127 and D on row 0.  A head-pair's U is thus
         assembled pair-stacked in SBUF [128, q] which makes the out
         projection a full contract-128 matmul (half the instructions).
  attn:  per head: S^T = K @ Q^T -> exp on ScalarE (x0.125 fused; no max
         subtraction, scores are O(1)) -> bf16 P^T -> PV accumulation.
         Denominator rows are staged via ScalarE copies + SBUF-to-SBUF
         DMA onto partitions {0,32,64,96} and reciprocal'd once per
         4-head group (the DVE reciprocal is free-dim-bound at 8
         cycles/elem, so batching heads onto partitions is 4x cheaper).
  out:   three passes (pairs 0-3 + bias, pairs 4-5, pairs 6-7)
         accumulating into resident SBUF tiles, so most of the out
         projection fills PE gaps during ScalarE-bound attention and the
         tail after the last head is short.
"""

import numpy as np
import ml_dtypes

B = 4
N = 2048
DIM = 1024
HEADS = 16
DH = 64
NQ = 1024  # query tokens per core
NCORES = 8

_CACHE = {}


def _build_nc():
    from contextlib import ExitStack

    import concourse.bass as bass
    import concourse.mybir as mybir
    import concourse.tile as tile
    from concourse import bacc

    f32 = mybir.dt.float32
    bf16 = mybir.dt.bfloat16
    EXP = mybir.ActivationFunctionType.Exp

    nc = bacc.Bacc("TRN2", target_bir_lowering=False, debug=False,
                   num_devices=NCORES)

    xt_d = nc.dram_tensor("xt", [DIM, N], bf16, kind="ExternalInput")
    wqkv_d = nc.dram_tensor("wqkv", [DIM, 3 * DIM], bf16, kind="ExternalInput")
    wo_d = nc.dram_tensor("wo", [HEADS // 2, 2 * DH, DIM], bf16,
                          kind="ExternalInput")
    brow_d = nc.dram_tensor("brow", [1, DIM], bf16, kind="ExternalInput")
    out_d = nc.dram_tensor("out", [NQ, DIM], f32, kind="ExternalOutput")

    with tile.TileContext(nc) as tc, ExitStack() as top:
        const_pool = top.enter_context(tc.tile_pool(name="const", bufs=1))
        mm_psum = top.enter_context(tc.tile_pool(name="mmps", bufs=2, space="PSUM"))
        sp_psum = top.enter_context(tc.tile_pool(name="spps", bufs=2, space="PSUM"))
        u_psum = top.enter_context(tc.tile_pool(name="ups", bufs=1, space="PSUM"))
        es_pool = top.enter_context(tc.tile_pool(name="es", bufs=6))
        dst_pool = top.enter_context(tc.tile_pool(name="dst", bufs=2))
        bc_pool = top.enter_context(tc.tile_pool(name="bc", bufs=3))
        uraw_a = top.enter_context(tc.tile_pool(name="uraw_a", bufs=1))

        brow_t = const_pool.tile([1, DIM], bf16, tag="brow", name="brow")
        nc.sync.dma_start(brow_t[:], brow_d.ap()[:])
        ones_t = const_pool.tile([1, 128], bf16, tag="ones", name="ones")
        nc.gpsimd.memset(ones_t[:], 1.0)
        # all-ones rows used by the per-pair normalizer broadcast matmuls;
        # only partitions {0,32,64,96} are used (contract-1 matmul rule).
        onesf = const_pool.tile([97, 128], f32, tag="onesf", name="onesf")
        nc.gpsimd.memset(onesf[:], 1.0)
        # denominator staging: head 4g+j's D row lands on partition 32*j.
        dsb_t = const_pool.tile([97, NQ], bf16, tag="dsb", name="dsb")
        rec_t = const_pool.tile([97, NQ], f32, tag="rec", name="rec")

        # HAM warm-up: ~6us of dependency-free matmuls on memset data so the
        # PE clock gate is at 8/8 before the first DMA-gated projection MM,
        # and the initial DMA wait doesn't re-throttle it.
        warm_t = const_pool.tile([1, 512], f32, tag="warm", name="warm")
        nc.gpsimd.memset(warm_t[:], 1.0)
        for _ in range(24):
            wps = mm_psum.tile([128, 512], f32, tag="mm", name="warm")
            nc.tensor.matmul(wps[:], onesf[0:1, 0:128], warm_t[:],
                             start=True, stop=True)

        ur2 = [None] * (HEADS // 2)  # pair-stacked raw U [128, NQ] bf16

        def proj_units(half, w_pool, xt, QT, KT, VTe, VTo):
            """Emission closures, one PSUM-group each.

            Order: all of V, then K/Q alternating per head-pair chunk so
            early head pairs become ready as soon as possible.
            """
            def dma_factory(col0):
                box = [None]
                def dma():
                    if box[0] is None:
                        wb = [w_pool.tile([128, 512], bf16, tag=f"w{fc}",
                                          name=f"w{fc}") for fc in range(8)]
                        for fc in range(8):
                            nc.sync.dma_start(
                                wb[fc][:],
                                wqkv_d.ap()[fc * 128:(fc + 1) * 128,
                                            col0:col0 + 512])
                        box[0] = wb
                    return box[0]
                return dma

            dma_v = dma_factory(2 * DIM + half * 512)
            dma_k = dma_factory(DIM + half * 512)
            dma_q = dma_factory(half * 512)

            def v_unit(mk, dma=dma_v):
                wb = dma()
                ps = mm_psum.tile([128, 512], f32, tag="mm", name="mm")
                for fc in range(8):
                    nc.tensor.matmul(
                        ps[:], xt[fc][:, mk * 128:(mk + 1) * 128], wb[fc][:],
                        start=(fc == 0), stop=(fc == 7))
                r = ps[:].rearrange("p (s two d) -> p s two d", two=2, d=64)
                nc.vector.tensor_copy(VTe[mk][:, :, 0:64], r[:, :, 0, :])
                nc.vector.tensor_copy(VTo[mk][:, :, 64:128], r[:, :, 1, :])

            def qk_unit(dma, dest, m4, t):
                wb = dma()
                ps = mm_psum.tile([128, 512], f32, tag="mm", name="mm")
                for fc in range(8):
                    nc.tensor.matmul(
                        ps[:], wb[fc][:, m4 * 128:(m4 + 1) * 128],
                        xt[fc][:, t * 512:(t + 1) * 512],
                        start=(fc == 0), stop=(fc == 7))
                nc.vector.tensor_copy(
                    dest[m4][:, t * 512:(t + 1) * 512], ps[:])

            units = [lambda mk=mk: v_unit(mk) for mk in range(16)]
            for m4 in range(4):
                for t in range(4):
                    units.append(lambda m4=m4, t=t: qk_unit(dma_k, KT, m4, t))
                for t in range(2):
                    units.append(lambda m4=m4, t=t: qk_unit(dma_q, QT, m4, t))
            return units, dma_v

        def emit_recip():
            """Whole-tile reciprocal of the staged denominators.

            Rows not refreshed since the last call just get the same
            values recomputed, so calling this after any subset of new
            gathers is idempotent for the rows later reads care about.
            """
            with nc.allow_low_precision(reason="softmax denom recip"):
                nc.vector.reciprocal(rec_t[:], dsb_t[:])

        def emit_norm_pair(p):
            """Normalize pair p's raw U by 1/D via a broadcast matmul
            (rows 0-63 even head, 64-127 odd head) + GpSimd multiply."""
            pp = p % 2
            re_row = 32 * (2 * pp)      # even head's rec_t partition
            ro_row = 32 * (2 * pp + 1)  # odd head's
            for qc in range(2):
                bc = mm_psum.tile([128, 512], f32, tag="mm", name="bc")
                nc.tensor.matmul(
                    bc[0:64, :], onesf[re_row:re_row + 1, 0:64],
                    rec_t[re_row:re_row + 1, qc * 512:(qc + 1) * 512],
                    start=True, stop=True,
                    tile_position=(re_row, 0))
                nc.tensor.matmul(
                    bc[64:128, :], onesf[ro_row:ro_row + 1, 64:128],
                    rec_t[ro_row:ro_row + 1, qc * 512:(qc + 1) * 512],
                    start=True, stop=True,
                    tile_position=(ro_row, 64))
                bc_sb = bc_pool.tile([128, 512], f32, tag="bc", name="bc_sb")
                nc.vector.tensor_copy(bc_sb[:], bc[:])
                nc.gpsimd.tensor_mul(
                    ur2[p][:, qc * 512:(qc + 1) * 512],
                    ur2[p][:, qc * 512:(qc + 1) * 512], bc_sb[:])

        dstate = {}
        # pipeline: a whole-tile reciprocal right after each pair's second
        # gather, and pair p's normalize ~2.5 heads later -- long before the
        # rec_t rows rotate to pair p+2 (at the recip after head 2p+5) and
        # with enough lead that the PE FIFO never head-blocks on the DVE.
        RECIP_AFTER = {1, 3, 5, 7, 9, 11, 13, 15}
        NORM_AT = {4: [0], 6: [1], 8: [2], 10: [3], 12: [4], 13: [5],
                   14: [6]}

        def emit_attn(heads, QTs, KTs, VTes, VTos, ur2_pool, fillers,
                      hooks=None):
            """Attention for the given heads; filler units spread across
            k-steps."""
            fillers = list(fillers)
            nfill = len(fillers)
            steps = len(heads) * 16
            done = 0
            for h in heads:
                if hooks and h in hooks:
                    hooks[h]()
                for p in NORM_AT.get(h, []):
                    emit_norm_pair(p)
                half = h // 8
                hh = h % 8
                QT, KT = QTs[half], KTs[half]
                pair = hh // 2
                hb = (hh % 2) * 64
                odd = h % 2
                slot = hh // 2
                Ups = u_psum.tile([128, 2, 512], f32, tag="up", name="up")
                for k in range(16):
                    sp = sp_psum.tile([128, 2, 512], f32, tag="sp", name="sp")
                    for qc in range(2):
                        nc.tensor.matmul(
                            sp[:, qc, :],
                            KT[pair][hb:hb + 64, k * 128:(k + 1) * 128],
                            QT[pair][hb:hb + 64, qc * 512:(qc + 1) * 512],
                            start=True, stop=True)
                    es = es_pool.tile([128, 2, 512], bf16, tag="es", name="es")
                    nc.scalar.activation(es[:], sp[:], EXP, scale=0.125)
                    for qc in range(2):
                        if odd:
                            nc.tensor.matmul(
                                Ups[:, qc, :],
                                VTos[half][k][:, slot, :],
                                es[:, qc, :],
                                start=(k == 0), stop=(k == 15))
                        else:
                            nc.tensor.matmul(
                                Ups[0:65, qc, :],
                                VTes[half][k][:, slot, :],
                                es[:, qc, :],
                                start=(k == 0), stop=(k == 15))
                    done += 1
                    while fillers and (nfill - len(fillers)) * steps < done * nfill:
                        fillers.pop(0)()
                # stash raw U into the pair-stacked SBUF tile (frees PSUM),
                # extract the denominator row via ScalarE, stage it for the
                # batched reciprocal via SBUF-to-SBUF DMA.
                p = h // 2
                if h % 4 == 0:
                    dstate["dst"] = dst_pool.tile([65, 2, NQ], bf16,
                                                  tag="dst", name="dst")
                dst = dstate["dst"]
                pslot = p % 2
                if ur2[p] is None:
                    ur2[p] = ur2_pool(h).tile([128, NQ], bf16, tag=f"ur{p}",
                                              name=f"ur{p}")
                if odd:
                    nc.vector.tensor_copy(
                        ur2[p][64:128, :].rearrange("p (a b) -> p a b", a=2),
                        Ups[64:128, :, :])
                    nc.scalar.copy(
                        dst[0:1, pslot, :].rearrange("p (a b) -> p a b", a=2),
                        Ups[0:1, :, :])
                    nc.sync.dma_start(
                        dsb_t[32 * (2 * pslot + 1):32 * (2 * pslot + 1) + 1, :],
                        dst[0:1, pslot, :])
                else:
                    nc.vector.tensor_copy(
                        ur2[p][0:64, :].rearrange("p (a b) -> p a b", a=2),
                        Ups[0:64, :, :])
                    nc.scalar.copy(
                        dst[64:65, pslot, :].rearrange("p (a b) -> p a b", a=2),
                        Ups[64:65, :, :])
                    nc.sync.dma_start(
                        dsb_t[32 * (2 * pslot):32 * (2 * pslot) + 1, :],
                        dst[64:65, pslot, :])
                if h in RECIP_AFTER:
                    emit_recip()
            for f in fillers:
                f()

        # ---------------- emission ----------------
        if True:
            xt_pool = tc.alloc_tile_pool(name="xt", bufs=1)
            w_pool = tc.alloc_tile_pool(name="w", bufs=2)
            xt = [xt_pool.tile([128, N], bf16, tag=f"xt{i}", name=f"xt{i}")
                  for i in range(8)]

            qkv0 = tc.alloc_tile_pool(name="qkv0", bufs=1)
            QT0 = [qkv0.tile([128, NQ], bf16, tag=f"q{m}", name=f"q0{m}")
                   for m in range(4)]
            KT0 = [qkv0.tile([128, N], bf16, tag=f"k{m}", name=f"k0{m}")
                   for m in range(4)]
            VTe0 = [qkv0.tile([128, 4, 65], bf16, tag=f"ve{mk}", name=f"ve0{mk}")
                    for mk in range(16)]
            VTo0 = [qkv0.tile([128, 4, 128], bf16, tag=f"vo{mk}", name=f"vo0{mk}")
                    for mk in range(16)]
            p0_units, p0_dma_v = proj_units(0, w_pool, xt, QT0, KT0, VTe0, VTo0)

            # weight blocks for proj-0 V first (small), then the x tiles in
            # token slices so the first v_unit only waits ~1.2 MB of DMA and
            # subsequent v_units stay ahead of the DMA stream.
            p0_dma_v()
            for lo, hi in ((0, 128), (128, 256), (256, 512), (512, 1024),
                           (1024, 1536), (1536, 2048)):
                for i in range(8):
                    nc.sync.dma_start(xt[i][:, lo:hi],
                                      xt_d.ap()[i * 128:(i + 1) * 128, lo:hi])

            # stationary layouts for the V matmuls: even heads carry the
            # softmax-denominator ones column at 64; odd heads put ones at
            # col 0 and V at 64-127 (U lands pair-stacked, D on row 0).
            for mk in range(16):
                nc.gpsimd.memset(VTe0[mk][:, :, 64:65], 1.0)
                nc.gpsimd.memset(VTo0[mk][:, :, 0:64], 0.0)
                nc.gpsimd.memset(VTo0[mk][:, :, 0:1], 1.0)

            # V + pair-0 K/Q serially (heads 0/1 cannot start without them;
            # Tile dependencies only look backward in emission order)
            for c in p0_units[:22]:
                c()
            p0_rest = p0_units[22:]

            qkv1 = tc.alloc_tile_pool(name="qkv1", bufs=1, side="right")
            QT1 = [qkv1.tile([128, NQ], bf16, tag=f"q{m}", name=f"q1{m}")
                   for m in range(4)]
            KT1 = [qkv1.tile([128, N], bf16, tag=f"k{m}", name=f"k1{m}")
                   for m in range(4)]
            VTe1 = [qkv1.tile([128, 4, 65], bf16, tag=f"ve{mk}", name=f"ve1{mk}")
                    for mk in range(16)]
            VTo1 = [qkv1.tile([128, 4, 128], bf16, tag=f"vo{mk}", name=f"vo1{mk}")
                    for mk in range(16)]
            for mk in range(16):
                nc.gpsimd.memset(VTe1[mk][:, :, 64:65], 1.0)
                nc.gpsimd.memset(VTo1[mk][:, :, 0:64], 0.0)
                nc.gpsimd.memset(VTo1[mk][:, :, 0:1], 1.0)
            p1_units, _ = proj_units(1, w_pool, xt, QT1, KT1, VTe1, VTo1)

            state = {}

            def setup_b():
                qkv0.release()
                state["uraw_b"] = tc.alloc_tile_pool(name="uraw_b", bufs=1,
                                                     side="right")
                wo_pool = tc.alloc_tile_pool(name="wo", bufs=1, side="right")
                state["wo_pool"] = wo_pool
                state["WO2"] = [wo_pool.tile([128, DIM], bf16, tag=f"wo{p}",
                                             name=f"wo{p}")
                                for p in range(HEADS // 2)]
                for p in range(HEADS // 2):
                    nc.sync.dma_start(state["WO2"][p][:], wo_d.ap()[p])

            def setup_c():
                # xt and the w-block tiles are dead once proj(1) is done
                w_pool.release()
                xt_pool.release()
                state["st_pool"] = tc.alloc_tile_pool(name="st", bufs=2)
                state["FIN"] = [
                    state["st_pool"].tile([128, DIM], f32, tag=f"fin{qf}",
                                          name=f"fin{qf}", bufs=1)
                    for qf in range(8)]

            # out-proj pass over `pairs` for one qf; first pass also folds
            # in the bias row and establishes the resident FIN tile.
            def out_pass(qf, pairs, first):
                WO2 = state["WO2"]
                fin = state["FIN"][qf]
                ps = [mm_psum.tile([128, 512], f32, tag="mm", name="mm")
                      for _ in range(2)]
                for i, p in enumerate(pairs):
                    last = (i == len(pairs) - 1) and not first
                    for of in range(2):
                        nc.tensor.matmul(
                            ps[of][:],
                            ur2[p][:, qf * 128:(qf + 1) * 128],
                            WO2[p][:, of * 512:(of + 1) * 512],
                            start=(i == 0), stop=last)
                if first:
                    # fold the bias row in as the accumulation-group closer
                    for of in range(2):
                        nc.tensor.matmul(
                            ps[of][:], ones_t[:, 0:128],
                            brow_t[:, of * 512:(of + 1) * 512],
                            start=False, stop=True)
                for of in range(2):
                    if first:
                        nc.vector.tensor_copy(fin[:, of * 512:(of + 1) * 512],
                                              ps[of][:])
                    else:
                        nc.vector.tensor_add(
                            fin[:, of * 512:(of + 1) * 512],
                            fin[:, of * 512:(of + 1) * 512], ps[of][:])

            ur2_pool = lambda h: (uraw_a if h < 8 else state["uraw_b"])

            # heads 0-9: remaining proj0 + all proj1 units fill PE gaps
            emit_attn(range(0, 10), [QT0, QT1], [KT0, KT1],
                      [VTe0, VTe1], [VTo0, VTo1], ur2_pool,
                      p0_rest + p1_units[:34], hooks={8: setup_b})
            # heads 10-13: rest of proj1 + pass A (pairs 0-3; normalized by
            # the norm emitted at head 10's start)
            emit_attn(range(10, 14), [QT0, QT1], [KT0, KT1],
                      [VTe0, VTe1], [VTo0, VTo1], ur2_pool,
                      p1_units[34:] +
                      [lambda qf=qf: out_pass(qf, [0, 1, 2, 3], True)
                       for qf in range(8)],
                      hooks={10: setup_c})
            # heads 14-15: pass B over pairs 4-6 (all normalized at head 14's
            # start thanks to the split pair-6 reciprocal after head 13)
            emit_attn(range(14, 16), [QT0, QT1], [KT0, KT1],
                      [VTe0, VTe1], [VTo0, VTo1], ur2_pool,
                      [lambda qf=qf: out_pass(qf, [4, 5, 6], False)
                       for qf in range(8)])

            # tail: only the last pair's normalize + pass C + writeback.
            emit_norm_pair(7)
            for qf in range(8):
                out_pass(qf, [7], False)
                nc.sync.dma_start(out_d.ap()[qf * 128:(qf + 1) * 128, :],
                                  state["FIN"][qf][:])

            state["st_pool"].release()
            state["wo_pool"].release()
            state["uraw_b"].release()
            qkv1.release()

    nc.compile()
    return nc


def _get_nc():
    if "nc" not in _CACHE:
        _CACHE["nc"] = _build_nc()
    return _CACHE["nc"]


def _make_in_maps(x, w_qkv, w_out, b_out):
    bf = ml_dtypes.bfloat16
    wo = np.ascontiguousarray(
        w_out.reshape(HEADS // 2, 2 * DH, DIM)).astype(bf)
    brow = np.asarray(b_out, np.float32).reshape(1, DIM).astype(bf)
    wqkv = np.ascontiguousarray(w_qkv, np.float32).astype(bf)
    in_maps = []
    for i in range(NCORES):
        b, half = i // 2, i % 2
        xt = np.asarray(x[b], np.float32).T.astype(bf)  # [DIM, N]
        if half:
            xt = np.concatenate([xt[:, NQ:], xt[:, :NQ]], axis=1)
        in_maps.append({
            "xt": np.ascontiguousarray(xt),
            "wqkv": wqkv,
            "wo": wo,
            "brow": brow,
        })
    return in_maps


def _assemble(results):
    out = np.empty((B, N, DIM), np.float32)
    for i in range(NCORES):
        b, half = i // 2, i % 2
        out[b, half * NQ:(half + 1) * NQ, :] = results[i]["out"]
    return out


def run(x, w_qkv, w_out, b_out, trace=False):
    """Run the kernel; returns (output, BassKernelResults)."""
    from concourse.bass_utils import run_bass_kernel_spmd
    nc = _get_nc()
    in_maps = _make_in_maps(x, w_qkv, w_out, b_out)
    res = run_bass_kernel_spmd(nc, in_maps, core_ids=list(range(NCORES)),
                               trace=trace)
    return _assemble(res.results), res


def kernel(x, w_qkv, w_out, b_out):
    out, _ = run(x, w_qkv, w_out, b_out, trace=False)
    return out
